# revision 13
# baseline (speedup 1.0000x reference)
"""Trainium2 Bass kernel for CustomPoseMixtureVAE (moe_routing).

Strategy: data-parallel over batch across 8 NeuronCores (256 rows/core),
all weights replicated, no collectives. Activations kept feature-major
[feat, batch] on-chip so every linear is a single PSUM-accumulated GEMM
chain with the (host-pretransposed) weight as the stationary operand.

The expert mixture  out = einsum('be,bi,eio->bo', coeff, inp, W)  is
computed as ONE GEMM over K = E*in using per-expert coefficient-scaled
inputs stacked along K:  out[b,o] = sum_{e,i} (coeff[b,e]*inp[b,i]) W[e,i,o].
Coefficient rows are broadcast across partitions with one-hot selector
matmuls on the PE; scaling is elementwise on DVE. The mixed bias
(coeff @ b_e) is folded in as an extra K-tile whose rhs is coeffT and
whose lhsT is the bias matrix.

ELU(x) = max(x, min(exp(x),1) - 1)   (exp monotonic => exp(min(x,0)) =
min(exp(x),1)); exp on ScalarE, the rest on Vector/GpSimd engines.
"""

import numpy as np
from contextlib import ExitStack

import concourse.bass as bass
import concourse.bacc as bacc
import concourse.tile as tile
import concourse.mybir as mybir
from concourse.bass_utils import run_bass_kernel_spmd

F16 = mybir.dt.float16
F32 = mybir.dt.float32
AOP = mybir.AluOpType
AF = mybir.ActivationFunctionType

B = 2048
NCORES = 8
BC = B // NCORES          # 256 batch rows per core
F = 267                   # frame size
L = 32                    # latent
H = 256                   # hidden
E = 8                     # experts
G = 64                    # gate hidden
IN0 = L + F               # 299
IN1 = L + H               # 288

LAST_RESULTS = None       # BassKernelResults of the most recent run
_CACHE = {}


def _eps42():
    """eps = jax.random.normal(key(42), (B, L)) exactly as the reference
    computes it, on the default jax backend (PRNG lowerings differ between
    backends, so we must mirror the reference's code path, not hardcode)."""
    if "eps" not in _CACHE:
        import jax
        import jax.numpy as jnp

        _CACHE["eps"] = np.asarray(
            jax.random.normal(jax.random.key(42), (B, L), jnp.float32)
        )
    return _CACHE["eps"]


def _pad_rows(a, rows):
    out = np.zeros((rows, a.shape[1]), a.dtype)
    out[: a.shape[0]] = a
    return out


def _bcast_ap(ap2d, reps):
    """[P, N] AP -> [P, reps, N] AP with a step-0 middle dim (operand bcast)."""
    return bass.AP(
        tensor=ap2d.tensor,
        offset=ap2d.offset,
        ap=[ap2d.ap[0], [0, reps], ap2d.ap[1]],
    )


def _build_program():
    nc = bacc.Bacc("TRN2")
    d = {}

    def din(name, shape, dt=F16):
        d[name] = nc.dram_tensor(name, shape, dt, kind="ExternalInput").ap()

    def dout(name, shape, dt=F32):
        d[name] = nc.dram_tensor(name, shape, dt, kind="ExternalOutput").ap()

    # per-core activations ([feat, batch] fp16; xo/co carry a trailing ones row)
    din("xo", [268, BC])
    din("co", [268, BC])
    din("epsT", [L, BC])
    # replicated weights (host-restacked, see kernel())
    din("sel", [8, 9, 128])
    din("fc1w", [6 * 128, 256])
    din("fc2w", [5 * 128, 256])
    din("mvw", [5 * 128, 64])
    din("g0w", [4 * 128, 64])
    din("g1w", [128, 64])
    din("g2w", [128, 8])
    din("w0st", [21 * 128, 256])
    din("w1st", [19 * 128, 256])
    din("w2st", [19 * 128, 267])
    dout("outT", [F, BC])
    dout("mvT", [2 * L, BC])

    with tile.TileContext(nc) as tc, ExitStack() as ctx:
        sb = ctx.enter_context(tc.tile_pool(name="sb", bufs=1))
        ps_big = ctx.enter_context(tc.tile_pool(name="ps_big", bufs=2, space="PSUM"))
        ps_sm = ctx.enter_context(tc.tile_pool(name="ps_sm", bufs=3, space="PSUM"))
        ps_bc = ctx.enter_context(tc.tile_pool(name="ps_bc", bufs=2, space="PSUM"))

        def sbt(tag, shape, dt=F16):
            return sb.tile(shape, dt, tag=tag, name=tag)

        # ---- input / weight DMAs -------------------------------------
        def dma_tiled(dst, src, nk, cols, eng=None):
            """DRAM [nk*128, cols] -> SBUF [128, nk, cols]."""
            eng = eng or nc.sync
            eng.dma_start(
                dst[:, 0:nk, :],
                src[0 : nk * 128, :].rearrange("(k p) m -> p k m", p=128),
            )

        sel = sbt("sel", [8, 9, 128])
        nc.sync.dma_start(sel[:], d["sel"][:])
        xo = sbt("xo", [128, 3, BC])
        nc.sync.dma_start(xo[:, 0:2, :], d["xo"][0:256, :].rearrange("(k p) n -> p k n", p=128))
        nc.sync.dma_start(xo[0:12, 2, :], d["xo"][256:268, :])
        co = sbt("co", [128, 3, BC])
        nc.sync.dma_start(co[:, 0:2, :], d["co"][0:256, :].rearrange("(k p) n -> p k n", p=128))
        nc.sync.dma_start(co[0:12, 2, :], d["co"][256:268, :])
        eps = sbt("eps", [L, BC])
        nc.sync.dma_start(eps[:], d["epsT"][:])

        fc1w = sbt("fc1w", [128, 6, 256])
        dma_tiled(fc1w, d["fc1w"], 6, 256)
        fc2w = sbt("fc2w", [128, 5, 256])
        dma_tiled(fc2w, d["fc2w"], 5, 256)
        mvw = sbt("mvw", [128, 5, 64])
        dma_tiled(mvw, d["mvw"], 5, 64)
        g0w = sbt("g0w", [128, 4, 64])
        dma_tiled(g0w, d["g0w"], 4, 64)
        g1w = sbt("g1w", [128, 64])
        nc.sync.dma_start(g1w[:], d["g1w"][:])
        g2w = sbt("g2w", [128, 8])
        nc.sync.dma_start(g2w[:], d["g2w"][:])

        # decoder weights: split DMAs so several queues pull in parallel
        w0 = sbt("w0", [128, 21, 256])
        w1 = sbt("w1", [128, 19, 256])
        w2 = sbt("w2", [128, 19, 267])
        for (wt, src, nk, cols) in ((w0, d["w0st"], 21, 256), (w1, d["w1st"], 19, 256), (w2, d["w2st"], 19, 267)):
            for lo in range(0, nk, 5):
                hi = min(lo + 5, nk)
                nc.sync.dma_start(
                    wt[:, lo:hi, :],
                    src[lo * 128 : hi * 128, :].rearrange("(k p) m -> p k m", p=128),
                )

        # L0 c-tail scaled tiles: zero the 21-row pads once, up front
        s0t = sbt("s0t", [128, 2, BC])
        nc.vector.memset(s0t[:], 0.0)

        # ---- helpers -------------------------------------------------
        def elu(pp, width, out_ap, tagbase):
            """ELU from psum pp [P, width] into fp16 out_ap (same shape)."""
            p = pp.shape[0]
            ex = sbt(tagbase + "_e", [p, width])
            nc.scalar.activation(ex[:], pp, AF.Exp)
            xl = sbt(tagbase + "_x", [p, width])
            nc.scalar.copy(xl[:], pp)
            tt = sbt(tagbase + "_t", [p, width])
            nc.vector.tensor_scalar(tt[:], ex[:], 1.0, 1.0, AOP.min, AOP.subtract)
            nc.vector.tensor_tensor(out_ap, xl[:], tt[:], AOP.max)

        # ---- encoder -------------------------------------------------
        with nc.named_scope("enc1"):
            ph1 = ps_big.tile([128, 512], F32, tag="pbig", name="pbig")
            enc1_rhs = [
                (xo[:, 0, :], slice(0, 128)),
                (xo[:, 1, :], slice(0, 128)),
                (xo[0:12, 2, :], slice(0, 12)),
                (co[:, 0, :], slice(0, 128)),
                (co[:, 1, :], slice(0, 128)),
                (co[0:12, 2, :], slice(0, 12)),
            ]
            for m in range(2):
                n = len(enc1_rhs)
                for i, (rh, krows) in enumerate(enc1_rhs):
                    nc.tensor.matmul(
                        ph1[:, m * BC : (m + 1) * BC],
                        fc1w[krows, i, m * 128 : (m + 1) * 128],
                        rh,
                        start=(i == 0),
                        stop=(i == n - 1),
                    )
            h1 = sbt("h1", [128, 2, BC])
            elu(ph1[:], 512, h1[:].rearrange("p k n -> p (k n)"), "eh1")

        with nc.named_scope("enc2"):
            ph2 = ps_big.tile([128, 512], F32, tag="pbig", name="pbig")
            enc2_rhs = [
                (xo[:, 0, :], slice(0, 128)),
                (xo[:, 1, :], slice(0, 128)),
                (xo[0:12, 2, :], slice(0, 12)),
                (h1[:, 0, :], slice(0, 128)),
                (h1[:, 1, :], slice(0, 128)),
            ]
            for m in range(2):
                n = len(enc2_rhs)
                for i, (rh, krows) in enumerate(enc2_rhs):
                    nc.tensor.matmul(
                        ph2[:, m * BC : (m + 1) * BC],
                        fc2w[krows, i, m * 128 : (m + 1) * 128],
                        rh,
                        start=(i == 0),
                        stop=(i == n - 1),
                    )
            h2 = sbt("h2", [128, 2, BC])
            elu(ph2[:], 512, h2[:].rearrange("p k n -> p (k n)"), "eh2")

        with nc.named_scope("muvar"):
            pmv = ps_sm.tile([64, BC], F32, tag="psm", name="psm")
            mv_rhs = [
                (xo[:, 0, :], slice(0, 128)),
                (xo[:, 1, :], slice(0, 128)),
                (xo[0:12, 2, :], slice(0, 12)),
                (h2[:, 0, :], slice(0, 128)),
                (h2[:, 1, :], slice(0, 128)),
            ]
            n = len(mv_rhs)
            for i, (rh, krows) in enumerate(mv_rhs):
                nc.tensor.matmul(pmv[:], mvw[krows, i, 0:64], rh, start=(i == 0), stop=(i == n - 1))
            mvf = sbt("mvf", [64, BC], F32)
            nc.vector.tensor_copy(mvf[:], pmv[:])          # fp32 mu/logvar out
            nc.sync.dma_start(d["mvT"][:], mvf[:])
            # z = mu + eps * exp(0.5*logvar)
            stdt = sbt("stdt", [L, BC])
            nc.scalar.activation(stdt[:], pmv[32:64, :], AF.Exp, scale=0.5)
            zt = sbt("zt", [L, BC])
            nc.vector.tensor_mul(zt[:], eps[:], stdt[:])
            z = sbt("z", [L, BC])
            nc.vector.tensor_add(z[:], zt[:], pmv[0:32, :])

        # ---- gate ----------------------------------------------------
        with nc.named_scope("gate"):
            g1a = sbt("g1a", [65, BC])
            g2a = sbt("g2a", [65, BC])
            nc.vector.memset(g1a[64:65, :], 1.0)
            nc.vector.memset(g2a[64:65, :], 1.0)
            pg0 = ps_sm.tile([64, BC], F32, tag="psm", name="psm")
            g0_rhs = [
                (z[:], slice(0, 32)),
                (co[:, 0, :], slice(0, 128)),
                (co[:, 1, :], slice(0, 128)),
                (co[0:12, 2, :], slice(0, 12)),
            ]
            n = len(g0_rhs)
            for i, (rh, krows) in enumerate(g0_rhs):
                nc.tensor.matmul(pg0[:], g0w[krows, i, 0:64], rh, start=(i == 0), stop=(i == n - 1))
            elu(pg0[:], BC, g1a[0:64, :], "eg0")

            pg1 = ps_sm.tile([64, BC], F32, tag="psm", name="psm")
            nc.tensor.matmul(pg1[:], g1w[0:65, 0:64], g1a[:], start=True, stop=True)
            elu(pg1[:], BC, g2a[0:64, :], "eg1")

            plg = ps_sm.tile([8, BC], F32, tag="psm", name="psm")
            nc.tensor.matmul(plg[:], g2w[0:65, 0:8], g2a[:], start=True, stop=True)

            # softmax over the 8 experts (partition dim) without max-sub
            expE = sbt("expE", [8, BC])
            nc.scalar.activation(expE[:], plg[:], AF.Exp)
            psum1 = ps_sm.tile([1, BC], F32, tag="psm", name="psm")
            nc.tensor.matmul(psum1[:], sel[0:8, 8, 0:1], expE[:], start=True, stop=True)
            recS = sbt("recS", [1, BC])
            with nc.allow_low_precision(reason="fp16 softmax denominators are well-conditioned here"):
                nc.vector.reciprocal(recS[:], psum1[:])
            pr8 = ps_sm.tile([8, BC], F32, tag="psm", name="psm")
            nc.tensor.matmul(pr8[:], sel[0:1, 8, 0:8], recS[:], start=True, stop=True)
            coeffT = sbt("coeffT", [8, BC])
            nc.vector.tensor_mul(coeffT[:], expE[:], pr8[:])

            # broadcast each coeff row to 128 partitions: sel_e.T @ coeffT
            bcastC = sbt("bcastC", [128, 8, BC])
            for i in range(4):
                pbc = ps_bc.tile([128, 512], F32, tag="pbc", name="pbc")
                nc.tensor.matmul(pbc[:, 0:BC], sel[:, 2 * i, :], coeffT[:], start=True, stop=True)
                nc.tensor.matmul(pbc[:, BC:512], sel[:, 2 * i + 1, :], coeffT[:], start=True, stop=True)
                nc.scalar.copy(bcastC[:, 2 * i : 2 * i + 2, :].rearrange("p k n -> p (k n)"), pbc[:])

        # ---- decoder -------------------------------------------------
        # z-part of the scaled stack (shared by all 3 layers)
        with nc.named_scope("zstack"):
            zs = sbt("zs", [128, 2, BC])
            for e in range(E):
                nc.vector.tensor_tensor(
                    zs[32 * (e % 4) : 32 * (e % 4) + 32, e // 4, :],
                    z[:],
                    bcastC[0:32, e, :],
                    AOP.mult,
                )

        def scaled_stack(name, src3, nsub):
            """Per-expert coefficient-scaled copies of src3 [128, nsub, BC]."""
            st = sbt(name, [128, E * nsub, BC])
            for e in range(E):
                nc.vector.tensor_tensor(
                    st[:, e * nsub : (e + 1) * nsub, :],
                    src3,
                    _bcast_ap(bcastC[:, e, :], nsub),
                    AOP.mult,
                )
            return st

        def decoder_layer(name, wt, stack_tiles, psum_w, mtiles, out_elu):
            """stack_tiles: list of (rhs_ap, krows) aligned with wt subtiles."""
            with nc.named_scope(name):
                pl = ps_big.tile([128, 512], F32, tag="pbig", name="pbig") if psum_w == 512 else None
                outs = []
                for mi, (mlo, mhi) in enumerate(mtiles):
                    if mhi - mlo == 128 and pl is not None:
                        pap = pl[:, mi * BC : (mi + 1) * BC]
                    else:
                        ptail = ps_sm.tile([mhi - mlo, BC], F32, tag="psm", name="psm")
                        pap = ptail[:]
                        outs.append(ptail)
                    n = len(stack_tiles)
                    for i, (rh, krows) in enumerate(stack_tiles):
                        nc.tensor.matmul(
                            pap,
                            wt[krows, i, mlo:mhi],
                            rh,
                            start=(i == 0),
                            stop=(i == n - 1),
                        )
                if out_elu is not None:
                    elu(pl[:], 512, out_elu[:].rearrange("p k n -> p (k n)"), "e" + name)
                return pl, outs

        with nc.named_scope("l0scale"):
            s0 = scaled_stack("s0", co[:, 0:2, :], 2)
            # c tail rows (11 per expert, padded to 32-partition blocks: compute
            # engines require 32-aligned partition bases)
            for e in range(E):
                nc.vector.tensor_tensor(
                    s0t[32 * (e % 4) : 32 * (e % 4) + 11, e // 4, :],
                    co[0:11, 2, :],
                    bcastC[0:11, e, :],
                    AOP.mult,
                )

        l0_tiles = [(zs[:, 0, :], slice(0, 128)), (zs[:, 1, :], slice(0, 128))]
        l0_tiles += [(s0[:, j, :], slice(0, 128)) for j in range(16)]
        l0_tiles += [(s0t[:, 0, :], slice(0, 128)), (s0t[:, 1, :], slice(0, 128)), (coeffT[:], slice(0, 8))]
        l0o = sbt("l0o", [128, 2, BC])
        decoder_layer("l0", w0, l0_tiles, 512, [(0, 128), (128, 256)], l0o)

        with nc.named_scope("l1scale"):
            s1 = scaled_stack("s1", l0o[:, 0:2, :], 2)
        l1_tiles = [(zs[:, 0, :], slice(0, 128)), (zs[:, 1, :], slice(0, 128))]
        l1_tiles += [(s1[:, j, :], slice(0, 128)) for j in range(16)]
        l1_tiles += [(coeffT[:], slice(0, 8))]
        l1o = sbt("l1o", [128, 2, BC])
        decoder_layer("l1", w1, l1_tiles, 512, [(0, 128), (128, 256)], l1o)

        with nc.named_scope("l2scale"):
            s2 = scaled_stack("s2", l1o[:, 0:2, :], 2)
        l2_tiles = [(zs[:, 0, :], slice(0, 128)), (zs[:, 1, :], slice(0, 128))]
        l2_tiles += [(s2[:, j, :], slice(0, 128)) for j in range(16)]
        l2_tiles += [(coeffT[:], slice(0, 8))]
        pl2, tails = decoder_layer(
            "l2", w2, l2_tiles, 512, [(0, 128), (128, 256), (256, 267)], None
        )

        with nc.named_scope("out"):
            out0 = sbt("out0", [128, 512], F32)
            nc.scalar.copy(out0[:], pl2[:])
            out2 = sbt("out2", [11, BC], F32)
            nc.scalar.copy(out2[:], tails[0][:])
            nc.sync.dma_start(d["outT"][0:128, :], out0[:, 0:BC])
            nc.sync.dma_start(d["outT"][128:256, :], out0[:, BC:512])
            nc.sync.dma_start(d["outT"][256:267, :], out2[:])

    nc.compile()
    return nc


def _host_weights(i):
    """Restack/transpose/cast all weights for the device layout."""
    f16 = np.float16

    def t(a):
        return np.asarray(a, np.float32).T  # [in, out]

    W1t = t(i["fc1_w"])  # [534, 256]
    fc1 = np.concatenate(
        [
            W1t[0:128],
            W1t[128:256],
            _pad_rows(np.concatenate([W1t[256:267], i["fc1_b"][None, :]], 0), 128),
            W1t[267:395],
            W1t[395:523],
            _pad_rows(W1t[523:534], 128),
        ],
        0,
    )
    W2t = t(i["fc2_w"])  # [523, 256]
    fc2 = np.concatenate(
        [
            W2t[0:128],
            W2t[128:256],
            _pad_rows(np.concatenate([W2t[256:267], i["fc2_b"][None, :]], 0), 128),
            W2t[267:395],
            W2t[395:523],
        ],
        0,
    )
    Wmv = np.concatenate([t(i["mu_w"]), t(i["lv_w"])], 1)  # [523, 64]
    bmv = np.concatenate([i["mu_b"], i["lv_b"]])[None, :]
    mv = np.concatenate(
        [
            Wmv[0:128],
            Wmv[128:256],
            _pad_rows(np.concatenate([Wmv[256:267], bmv], 0), 128),
            Wmv[267:395],
            Wmv[395:523],
        ],
        0,
    )
    G0 = t(i["g0_w"])  # [299, 64]
    g0 = np.concatenate(
        [
            _pad_rows(G0[0:32], 128),
            G0[32:160],
            G0[160:288],
            _pad_rows(np.concatenate([G0[288:299], i["g0_b"][None, :]], 0), 128),
        ],
        0,
    )
    g1 = _pad_rows(np.concatenate([t(i["g1_w"]), i["g1_b"][None, :]], 0), 128)
    g2 = _pad_rows(np.concatenate([t(i["g2_w"]), i["g2_b"][None, :]], 0), 128)

    def dec_stack(w, b):
        w = np.asarray(w, np.float32)  # [E, in, out]
        parts = [np.concatenate([w[e, 0:32] for e in range(4)], 0)]
        parts.append(np.concatenate([w[e, 0:32] for e in range(4, 8)], 0))
        for e in range(E):
            parts.append(w[e, 32:160])
            parts.append(w[e, 160:288])
        if w.shape[1] == IN0:  # layer 0: c tail rows, 32-row block per expert
            for g in range(2):
                parts.append(
                    np.concatenate(
                        [_pad_rows(w[e, 288:299], 32) for e in range(4 * g, 4 * g + 4)], 0
                    )
                )
        parts.append(_pad_rows(np.asarray(b, np.float32), 128))
        return np.concatenate(parts, 0)

    w0st = dec_stack(i["w0"], i["b0"])
    w1st = dec_stack(i["w1"], i["b1"])
    w2st = dec_stack(i["w2"], i["b2"])

    sel = np.zeros((8, 9, 128), np.float32)
    for e in range(E):
        sel[e, e, :] = 1.0
    sel[:, 8, :] = 1.0

    return {
        "sel": sel.astype(f16),
        "fc1w": fc1.astype(f16),
        "fc2w": fc2.astype(f16),
        "mvw": mv.astype(f16),
        "g0w": g0.astype(f16),
        "g1w": g1.astype(f16),
        "g2w": g2.astype(f16),
        "w0st": w0st.astype(f16),
        "w1st": w1st.astype(f16),
        "w2st": w2st.astype(f16),
    }


def kernel(**inputs):
    global LAST_RESULTS
    if "nc" not in _CACHE:
        _CACHE["nc"] = _build_program()
    nc = _CACHE["nc"]

    i = {k: np.asarray(v) for k, v in inputs.items()}
    eps = _eps42()
    wmap = _host_weights(i)

    ones = np.ones((1, B), np.float32)
    xo_full = np.concatenate([np.asarray(i["x"], np.float32).T, ones], 0).astype(np.float16)
    co_full = np.concatenate([np.asarray(i["c"], np.float32).T, ones], 0).astype(np.float16)
    epsT = eps.T.astype(np.float16)

    in_maps = []
    for ci in range(NCORES):
        s = slice(ci * BC, (ci + 1) * BC)
        m = dict(wmap)
        m["xo"] = np.ascontiguousarray(xo_full[:, s])
        m["co"] = np.ascontiguousarray(co_full[:, s])
        m["epsT"] = np.ascontiguousarray(epsT[:, s])
        in_maps.append(m)

    res = run_bass_kernel_spmd(nc, in_maps, core_ids=list(range(NCORES)))
    LAST_RESULTS = res

    out = np.empty((B, F), np.float32)
    mu = np.empty((B, L), np.float32)
    lv = np.empty((B, L), np.float32)
    for ci in range(NCORES):
        s = slice(ci * BC, (ci + 1) * BC)
        r = res.results[ci]
        out[s] = r["outT"].T
        mu[s] = r["mvT"][0:L].T
        lv[s] = r["mvT"][L : 2 * L].T
    return out, mu, lv


# revision 20
# speedup vs baseline: 1.0453x; 1.0453x over previous
"""Trainium2 Bass kernel for CustomPoseMixtureVAE (moe_routing).

Strategy: data-parallel over batch across 8 NeuronCores (256 rows/core),
all weights replicated, no collectives. Activations kept feature-major
[feat, batch] on-chip so every linear is a single PSUM-accumulated GEMM
chain with the (host-pretransposed) weight as the stationary operand.

The expert mixture  out = einsum('be,bi,eio->bo', coeff, inp, W)  is
computed as ONE GEMM over K = E*in using per-expert coefficient-scaled
inputs stacked along K:  out[b,o] = sum_{e,i} (coeff[b,e]*inp[b,i]) W[e,i,o].
Coefficient rows are broadcast across partitions with one-hot selector
matmuls on the PE; scaling is elementwise on DVE. The mixed bias
(coeff @ b_e) is folded in as an extra K-tile whose rhs is coeffT and
whose lhsT is the bias matrix.

ELU(x) = max(x, min(exp(x),1) - 1)   (exp monotonic => exp(min(x,0)) =
min(exp(x),1)); exp on ScalarE, the rest on Vector/GpSimd engines.
"""

import numpy as np
from contextlib import ExitStack

import concourse.bass as bass
import concourse.bacc as bacc
import concourse.tile as tile
import concourse.mybir as mybir
from concourse.bass_utils import run_bass_kernel_spmd

F16 = mybir.dt.float16
F32 = mybir.dt.float32
AOP = mybir.AluOpType
AF = mybir.ActivationFunctionType

B = 2048
NCORES = 8
BC = B // NCORES          # 256 batch rows per core
F = 267                   # frame size
L = 32                    # latent
H = 256                   # hidden
E = 8                     # experts
G = 64                    # gate hidden
IN0 = L + F               # 299
IN1 = L + H               # 288

LAST_RESULTS = None       # BassKernelResults of the most recent run
_CACHE = {}


def _eps42():
    """eps = jax.random.normal(key(42), (B, L)) exactly as the reference
    computes it, on the default jax backend (PRNG lowerings differ between
    backends, so we must mirror the reference's code path, not hardcode)."""
    if "eps" not in _CACHE:
        import jax
        import jax.numpy as jnp

        _CACHE["eps"] = np.asarray(
            jax.random.normal(jax.random.key(42), (B, L), jnp.float32)
        )
    return _CACHE["eps"]


def _pad_rows(a, rows):
    out = np.zeros((rows, a.shape[1]), a.dtype)
    out[: a.shape[0]] = a
    return out


def _bcast_ap(ap2d, reps):
    """[P, N] AP -> [P, reps, N] AP with a step-0 middle dim (operand bcast)."""
    return bass.AP(
        tensor=ap2d.tensor,
        offset=ap2d.offset,
        ap=[ap2d.ap[0], [0, reps], ap2d.ap[1]],
    )


def _build_program():
    nc = bacc.Bacc("TRN2")
    d = {}

    def din(name, shape, dt=F16):
        d[name] = nc.dram_tensor(name, shape, dt, kind="ExternalInput").ap()

    def dout(name, shape, dt=F32):
        d[name] = nc.dram_tensor(name, shape, dt, kind="ExternalOutput").ap()

    # per-core activations ([feat, batch] fp16; xo/co carry a trailing ones row)
    din("xo", [268, BC])
    din("co", [268, BC])
    din("epsT", [L, BC])
    # replicated weights (host-restacked, see kernel())
    din("sel", [8, 9, 128])
    din("ident", [128, 128], F32)
    din("fc1w", [6 * 128, 256])
    din("fc2w", [5 * 128, 256])
    din("mvw", [5 * 128, 64])
    din("g0w", [4 * 128, 64])
    din("g1w", [128, 64])
    din("g2w", [128, 8])
    din("w0st", [21 * 128, 256])
    din("w1st", [19 * 128, 256])
    din("w2st", [19 * 128, 267])
    dout("outT", [F, BC])
    dout("mvT", [2 * L, BC])

    with tile.TileContext(nc) as tc, ExitStack() as ctx:
        sb = ctx.enter_context(tc.tile_pool(name="sb", bufs=1))
        ps_big = ctx.enter_context(tc.tile_pool(name="ps_big", bufs=2, space="PSUM"))
        ps_sm = ctx.enter_context(tc.tile_pool(name="ps_sm", bufs=3, space="PSUM"))
        ps_bc = ctx.enter_context(tc.tile_pool(name="ps_bc", bufs=2, space="PSUM"))

        def sbt(tag, shape, dt=F16):
            return sb.tile(shape, dt, tag=tag, name=tag)

        # ---- input / weight DMAs -------------------------------------
        def dma_tiled(dst, src, nk, cols, eng=None):
            """DRAM [nk*128, cols] -> SBUF [128, nk, cols]."""
            eng = eng or nc.sync
            eng.dma_start(
                dst[:, 0:nk, :],
                src[0 : nk * 128, :].rearrange("(k p) m -> p k m", p=128),
            )

        # Spread DMAs over all five engines' queues so they pull in parallel,
        # first-needed tensors first on each queue.
        engs = [nc.sync, nc.scalar, nc.gpsimd]

        xo = sbt("xo", [128, 3, BC])
        nc.sync.dma_start(xo[:, 0:2, :], d["xo"][0:256, :].rearrange("(k p) n -> p k n", p=128))
        nc.gpsimd.dma_start(xo[0:12, 2, :], d["xo"][256:268, :])
        co = sbt("co", [128, 3, BC])
        nc.scalar.dma_start(co[:, 0:2, :], d["co"][0:256, :].rearrange("(k p) n -> p k n", p=128))
        nc.gpsimd.dma_start(co[0:12, 2, :], d["co"][256:268, :])
        fc1w = sbt("fc1w", [128, 6, 256])
        nc.sync.dma_start(fc1w[:, 0:3, :], d["fc1w"][0:384, :].rearrange("(k p) m -> p k m", p=128))
        nc.scalar.dma_start(fc1w[:, 3:6, :], d["fc1w"][384:768, :].rearrange("(k p) m -> p k m", p=128))
        eps = sbt("eps", [L, BC])
        nc.gpsimd.dma_start(eps[:], d["epsT"][:])
        sel = sbt("sel", [8, 9, 128])
        nc.gpsimd.dma_start(sel[:], d["sel"][:])
        ident = sbt("ident", [128, 128], F32)
        nc.gpsimd.dma_start(ident[:], d["ident"][:])

        fc2w = sbt("fc2w", [128, 5, 256])
        dma_tiled(fc2w, d["fc2w"], 5, 256, eng=nc.sync)
        mvw = sbt("mvw", [128, 5, 64])
        dma_tiled(mvw, d["mvw"], 5, 64, eng=nc.scalar)
        g0w = sbt("g0w", [128, 4, 64])
        dma_tiled(g0w, d["g0w"], 4, 64, eng=nc.gpsimd)
        g1w = sbt("g1w", [128, 64])
        nc.gpsimd.dma_start(g1w[:], d["g1w"][:])
        g2w = sbt("g2w", [128, 8])
        nc.gpsimd.dma_start(g2w[:], d["g2w"][:])

        # decoder weights: chunked + striped over all queues
        w0 = sbt("w0", [128, 21, 256])
        w1 = sbt("w1", [128, 19, 256])
        w2 = sbt("w2", [128, 19, 267])
        qi = 0
        for (wt, src, nk) in ((w0, d["w0st"], 21), (w1, d["w1st"], 19), (w2, d["w2st"], 19)):
            for lo in range(0, nk, 4):
                hi = min(lo + 4, nk)
                engs[qi % len(engs)].dma_start(
                    wt[:, lo:hi, :],
                    src[lo * 128 : hi * 128, :].rearrange("(k p) m -> p k m", p=128),
                )
                qi += 1

        # L0 c-tail scaled tiles: zero the 21-row pads once, up front
        s0t = sbt("s0t", [128, 2, BC])
        nc.vector.memset(s0t[:], 0.0)

        # ---- helpers -------------------------------------------------
        def elu(pp, width, out_ap, tagbase):
            """ELU(x) = relu(x) + (min(exp(x),1) - 1), x = psum pp [P, width]."""
            p = pp.shape[0]
            ex = sbt(tagbase + "_e", [p, width])
            nc.scalar.activation(ex[:], pp, AF.Exp)
            tt = sbt(tagbase + "_t", [p, width])
            nc.vector.tensor_scalar(tt[:], ex[:], 1.0, 1.0, AOP.min, AOP.subtract)
            nc.vector.scalar_tensor_tensor(out_ap, pp, 0.0, tt[:], AOP.max, AOP.add)

        # ---- encoder -------------------------------------------------
        with nc.named_scope("enc1"):
            ph1 = ps_big.tile([128, 512], F32, tag="pbig", name="pbig")
            enc1_rhs = [
                (xo[:, 0, :], slice(0, 128)),
                (xo[:, 1, :], slice(0, 128)),
                (xo[0:12, 2, :], slice(0, 12)),
                (co[:, 0, :], slice(0, 128)),
                (co[:, 1, :], slice(0, 128)),
                (co[0:12, 2, :], slice(0, 12)),
            ]
            for m in range(2):
                n = len(enc1_rhs)
                for i, (rh, krows) in enumerate(enc1_rhs):
                    nc.tensor.matmul(
                        ph1[:, m * BC : (m + 1) * BC],
                        fc1w[krows, i, m * 128 : (m + 1) * 128],
                        rh,
                        start=(i == 0),
                        stop=(i == n - 1),
                    )
            h1 = sbt("h1", [128, 2, BC])
            elu(ph1[:], 512, h1[:].rearrange("p k n -> p (k n)"), "eh1")

        with nc.named_scope("enc2"):
            ph2 = ps_big.tile([128, 512], F32, tag="pbig", name="pbig")
            enc2_rhs = [
                (xo[:, 0, :], slice(0, 128)),
                (xo[:, 1, :], slice(0, 128)),
                (xo[0:12, 2, :], slice(0, 12)),
                (h1[:, 0, :], slice(0, 128)),
                (h1[:, 1, :], slice(0, 128)),
            ]
            for m in range(2):
                n = len(enc2_rhs)
                for i, (rh, krows) in enumerate(enc2_rhs):
                    nc.tensor.matmul(
                        ph2[:, m * BC : (m + 1) * BC],
                        fc2w[krows, i, m * 128 : (m + 1) * 128],
                        rh,
                        start=(i == 0),
                        stop=(i == n - 1),
                    )
            h2 = sbt("h2", [128, 2, BC])
            elu(ph2[:], 512, h2[:].rearrange("p k n -> p (k n)"), "eh2")

        with nc.named_scope("muvar"):
            pmv = ps_sm.tile([64, BC], F32, tag="psm", name="psm")
            mv_rhs = [
                (xo[:, 0, :], slice(0, 128)),
                (xo[:, 1, :], slice(0, 128)),
                (xo[0:12, 2, :], slice(0, 12)),
                (h2[:, 0, :], slice(0, 128)),
                (h2[:, 1, :], slice(0, 128)),
            ]
            n = len(mv_rhs)
            for i, (rh, krows) in enumerate(mv_rhs):
                nc.tensor.matmul(pmv[:], mvw[krows, i, 0:64], rh, start=(i == 0), stop=(i == n - 1))
            mvf = sbt("mvf", [64, BC], F32)
            nc.scalar.copy(mvf[:], pmv[:])                 # fp32 mu/logvar out
            nc.sync.dma_start(d["mvT"][:], mvf[:])
            # z = mu + eps * exp(0.5*logvar)
            stdt = sbt("stdt", [L, BC])
            nc.scalar.activation(stdt[:], pmv[32:64, :], AF.Exp, scale=0.5)
            zt = sbt("zt", [L, BC])
            nc.vector.tensor_mul(zt[:], eps[:], stdt[:])
            z = sbt("z", [L, BC])
            nc.vector.tensor_add(z[:], zt[:], pmv[0:32, :])

        # ---- gate ----------------------------------------------------
        with nc.named_scope("gate"):
            g1a = sbt("g1a", [65, BC])
            g2a = sbt("g2a", [65, BC])
            nc.vector.memset(g1a[64:65, :], 1.0)
            nc.vector.memset(g2a[64:65, :], 1.0)
            pg0 = ps_sm.tile([64, BC], F32, tag="psm", name="psm")
            g0_rhs = [
                (z[:], slice(0, 32)),
                (co[:, 0, :], slice(0, 128)),
                (co[:, 1, :], slice(0, 128)),
                (co[0:12, 2, :], slice(0, 12)),
            ]
            n = len(g0_rhs)
            for i, (rh, krows) in enumerate(g0_rhs):
                nc.tensor.matmul(pg0[:], g0w[krows, i, 0:64], rh, start=(i == 0), stop=(i == n - 1))
            elu(pg0[:], BC, g1a[0:64, :], "eg0")

            pg1 = ps_sm.tile([64, BC], F32, tag="psm", name="psm")
            nc.tensor.matmul(pg1[:], g1w[0:65, 0:64], g1a[:], start=True, stop=True)
            elu(pg1[:], BC, g2a[0:64, :], "eg1")

            plg = ps_sm.tile([8, BC], F32, tag="psm", name="psm")
            nc.tensor.matmul(plg[:], g2w[0:65, 0:8], g2a[:], start=True, stop=True)

            # softmax over the 8 experts (partition dim) without max-sub.
            # The 1/sum is computed batch-on-partitions so the DVE iterative
            # divide runs 2 elems/lane x 128 lanes instead of 256 on one lane.
            expE = sbt("expE", [8, BC])
            nc.scalar.activation(expE[:], plg[:], AF.Exp)
            pst = ps_sm.tile([128, 2], F32, tag="psm", name="psm")
            nc.tensor.matmul(pst[:, 0:1], expE[0:8, 0:128], sel[0:8, 8, 0:1], start=True, stop=True)
            nc.tensor.matmul(pst[:, 1:2], expE[0:8, 128:256], sel[0:8, 8, 0:1], start=True, stop=True)
            rst = sbt("rst", [128, 2], F32)
            with nc.allow_low_precision(reason="softmax denominators are well-conditioned"):
                nc.vector.reciprocal(rst[:], pst[:])
            precS = ps_sm.tile([1, BC], F32, tag="psm", name="psm")
            nc.tensor.matmul(precS[0:1, 0:128], rst[:, 0:1], ident[:], start=True, stop=True)
            nc.tensor.matmul(precS[0:1, 128:256], rst[:, 1:2], ident[:], start=True, stop=True)
            recS = sbt("recS", [1, BC])
            nc.scalar.copy(recS[:], precS[:])
            pr8 = ps_sm.tile([8, BC], F32, tag="psm", name="psm")
            nc.tensor.matmul(pr8[:], sel[0:1, 8, 0:8], recS[:], start=True, stop=True)
            coeffT = sbt("coeffT", [8, BC])
            nc.vector.tensor_mul(coeffT[:], expE[:], pr8[:])

            # broadcast each coeff row to 128 partitions: sel_e.T @ coeffT
            bcastC = sbt("bcastC", [128, 8, BC])
            for i in range(4):
                pbc = ps_bc.tile([128, 512], F32, tag="pbc", name="pbc")
                nc.tensor.matmul(pbc[:, 0:BC], sel[:, 2 * i, :], coeffT[:], start=True, stop=True)
                nc.tensor.matmul(pbc[:, BC:512], sel[:, 2 * i + 1, :], coeffT[:], start=True, stop=True)
                nc.scalar.copy(bcastC[:, 2 * i : 2 * i + 2, :].rearrange("p k n -> p (k n)"), pbc[:])

        # ---- decoder -------------------------------------------------
        # z-part of the scaled stack (shared by all 3 layers)
        with nc.named_scope("zstack"):
            zs = sbt("zs", [128, 2, BC])
            for e in range(E):
                nc.vector.tensor_tensor(
                    zs[32 * (e % 4) : 32 * (e % 4) + 32, e // 4, :],
                    z[:],
                    bcastC[0:32, e, :],
                    AOP.mult,
                )

        def scaled_stack(name, src3, nsub):
            """Per-expert coefficient-scaled copies of src3 [128, nsub, BC]."""
            st = sbt(name, [128, E * nsub, BC])
            for e in range(E):
                nc.vector.tensor_tensor(
                    st[:, e * nsub : (e + 1) * nsub, :],
                    src3,
                    _bcast_ap(bcastC[:, e, :], nsub),
                    AOP.mult,
                )
            return st

        def decoder_layer(name, wt, stack_tiles, psum_w, mtiles, out_elu):
            """stack_tiles: list of (rhs_ap, krows) aligned with wt subtiles."""
            with nc.named_scope(name):
                pl = ps_big.tile([128, 512], F32, tag="pbig", name="pbig") if psum_w == 512 else None
                outs = []
                for mi, (mlo, mhi) in enumerate(mtiles):
                    if mhi - mlo == 128 and pl is not None:
                        pap = pl[:, mi * BC : (mi + 1) * BC]
                    else:
                        ptail = ps_sm.tile([mhi - mlo, BC], F32, tag="psm", name="psm")
                        pap = ptail[:]
                        outs.append(ptail)
                    n = len(stack_tiles)
                    for i, (rh, krows) in enumerate(stack_tiles):
                        nc.tensor.matmul(
                            pap,
                            wt[krows, i, mlo:mhi],
                            rh,
                            start=(i == 0),
                            stop=(i == n - 1),
                        )
                if out_elu is not None:
                    elu(pl[:], 512, out_elu[:].rearrange("p k n -> p (k n)"), "e" + name)
                return pl, outs

        with nc.named_scope("l0scale"):
            s0 = scaled_stack("s0", co[:, 0:2, :], 2)
            # c tail rows (11 per expert, padded to 32-partition blocks: compute
            # engines require 32-aligned partition bases)
            for e in range(E):
                nc.vector.tensor_tensor(
                    s0t[32 * (e % 4) : 32 * (e % 4) + 11, e // 4, :],
                    co[0:11, 2, :],
                    bcastC[0:11, e, :],
                    AOP.mult,
                )

        l0_tiles = [(zs[:, 0, :], slice(0, 128)), (zs[:, 1, :], slice(0, 128))]
        l0_tiles += [(s0[:, j, :], slice(0, 128)) for j in range(16)]
        l0_tiles += [(s0t[:, 0, :], slice(0, 128)), (s0t[:, 1, :], slice(0, 128)), (coeffT[:], slice(0, 8))]
        l0o = sbt("l0o", [128, 2, BC])
        decoder_layer("l0", w0, l0_tiles, 512, [(0, 128), (128, 256)], l0o)

        with nc.named_scope("l1scale"):
            s1 = scaled_stack("s1", l0o[:, 0:2, :], 2)
        l1_tiles = [(zs[:, 0, :], slice(0, 128)), (zs[:, 1, :], slice(0, 128))]
        l1_tiles += [(s1[:, j, :], slice(0, 128)) for j in range(16)]
        l1_tiles += [(coeffT[:], slice(0, 8))]
        l1o = sbt("l1o", [128, 2, BC])
        decoder_layer("l1", w1, l1_tiles, 512, [(0, 128), (128, 256)], l1o)

        with nc.named_scope("l2scale"):
            s2 = scaled_stack("s2", l1o[:, 0:2, :], 2)
        l2_tiles = [(zs[:, 0, :], slice(0, 128)), (zs[:, 1, :], slice(0, 128))]
        l2_tiles += [(s2[:, j, :], slice(0, 128)) for j in range(16)]
        l2_tiles += [(coeffT[:], slice(0, 8))]
        pl2, tails = decoder_layer(
            "l2", w2, l2_tiles, 512, [(0, 128), (128, 256), (256, 267)], None
        )

        with nc.named_scope("out"):
            out0 = sbt("out0", [128, 512], F32)
            nc.scalar.copy(out0[:], pl2[:])
            out2 = sbt("out2", [11, BC], F32)
            nc.scalar.copy(out2[:], tails[0][:])
            nc.sync.dma_start(d["outT"][0:128, :], out0[:, 0:BC])
            nc.sync.dma_start(d["outT"][128:256, :], out0[:, BC:512])
            nc.sync.dma_start(d["outT"][256:267, :], out2[:])

    nc.compile()
    return nc


def _host_weights(i):
    """Restack/transpose/cast all weights for the device layout."""
    f16 = np.float16

    def t(a):
        return np.asarray(a, np.float32).T  # [in, out]

    W1t = t(i["fc1_w"])  # [534, 256]
    fc1 = np.concatenate(
        [
            W1t[0:128],
            W1t[128:256],
            _pad_rows(np.concatenate([W1t[256:267], i["fc1_b"][None, :]], 0), 128),
            W1t[267:395],
            W1t[395:523],
            _pad_rows(W1t[523:534], 128),
        ],
        0,
    )
    W2t = t(i["fc2_w"])  # [523, 256]
    fc2 = np.concatenate(
        [
            W2t[0:128],
            W2t[128:256],
            _pad_rows(np.concatenate([W2t[256:267], i["fc2_b"][None, :]], 0), 128),
            W2t[267:395],
            W2t[395:523],
        ],
        0,
    )
    Wmv = np.concatenate([t(i["mu_w"]), t(i["lv_w"])], 1)  # [523, 64]
    bmv = np.concatenate([i["mu_b"], i["lv_b"]])[None, :]
    mv = np.concatenate(
        [
            Wmv[0:128],
            Wmv[128:256],
            _pad_rows(np.concatenate([Wmv[256:267], bmv], 0), 128),
            Wmv[267:395],
            Wmv[395:523],
        ],
        0,
    )
    G0 = t(i["g0_w"])  # [299, 64]
    g0 = np.concatenate(
        [
            _pad_rows(G0[0:32], 128),
            G0[32:160],
            G0[160:288],
            _pad_rows(np.concatenate([G0[288:299], i["g0_b"][None, :]], 0), 128),
        ],
        0,
    )
    g1 = _pad_rows(np.concatenate([t(i["g1_w"]), i["g1_b"][None, :]], 0), 128)
    g2 = _pad_rows(np.concatenate([t(i["g2_w"]), i["g2_b"][None, :]], 0), 128)

    def dec_stack(w, b):
        w = np.asarray(w, np.float32)  # [E, in, out]
        parts = [np.concatenate([w[e, 0:32] for e in range(4)], 0)]
        parts.append(np.concatenate([w[e, 0:32] for e in range(4, 8)], 0))
        for e in range(E):
            parts.append(w[e, 32:160])
            parts.append(w[e, 160:288])
        if w.shape[1] == IN0:  # layer 0: c tail rows, 32-row block per expert
            for g in range(2):
                parts.append(
                    np.concatenate(
                        [_pad_rows(w[e, 288:299], 32) for e in range(4 * g, 4 * g + 4)], 0
                    )
                )
        parts.append(_pad_rows(np.asarray(b, np.float32), 128))
        return np.concatenate(parts, 0)

    w0st = dec_stack(i["w0"], i["b0"])
    w1st = dec_stack(i["w1"], i["b1"])
    w2st = dec_stack(i["w2"], i["b2"])

    sel = np.zeros((8, 9, 128), np.float32)
    for e in range(E):
        sel[e, e, :] = 1.0
    sel[:, 8, :] = 1.0

    return {
        "ident": np.eye(128, dtype=np.float32),
        "sel": sel.astype(f16),
        "fc1w": fc1.astype(f16),
        "fc2w": fc2.astype(f16),
        "mvw": mv.astype(f16),
        "g0w": g0.astype(f16),
        "g1w": g1.astype(f16),
        "g2w": g2.astype(f16),
        "w0st": w0st.astype(f16),
        "w1st": w1st.astype(f16),
        "w2st": w2st.astype(f16),
    }


def kernel(**inputs):
    global LAST_RESULTS
    if "nc" not in _CACHE:
        _CACHE["nc"] = _build_program()
    nc = _CACHE["nc"]

    i = {k: np.asarray(v) for k, v in inputs.items()}
    eps = _eps42()
    wmap = _host_weights(i)

    ones = np.ones((1, B), np.float32)
    xo_full = np.concatenate([np.asarray(i["x"], np.float32).T, ones], 0).astype(np.float16)
    co_full = np.concatenate([np.asarray(i["c"], np.float32).T, ones], 0).astype(np.float16)
    epsT = eps.T.astype(np.float16)

    in_maps = []
    for ci in range(NCORES):
        s = slice(ci * BC, (ci + 1) * BC)
        m = dict(wmap)
        m["xo"] = np.ascontiguousarray(xo_full[:, s])
        m["co"] = np.ascontiguousarray(co_full[:, s])
        m["epsT"] = np.ascontiguousarray(epsT[:, s])
        in_maps.append(m)

    res = run_bass_kernel_spmd(nc, in_maps, core_ids=list(range(NCORES)))
    LAST_RESULTS = res

    out = np.empty((B, F), np.float32)
    mu = np.empty((B, L), np.float32)
    lv = np.empty((B, L), np.float32)
    for ci in range(NCORES):
        s = slice(ci * BC, (ci + 1) * BC)
        r = res.results[ci]
        out[s] = r["outT"].T
        mu[s] = r["mvT"][0:L].T
        lv[s] = r["mvT"][L : 2 * L].T
    return out, mu, lv
